# revision 11
# baseline (speedup 1.0000x reference)
"""PolyLoRALinear Trainium2 kernel (8-core SPMD, data-parallel over batch).

Per-core program (core c handles batch c):
  routing: w = sigmoid(module_logits[task_id]); w /= (sum(w) + eps)
  A_mix(d,16) = sum_s w_s * lora_a[s], B_mix(16,o) = sum_s w_s * lora_b[s]
  xA^T(16,t)  = A_mix^T @ x^T                       (1/16 folded into A path)
  y^T(o,t)    = W @ x^T + bias + B_mix^T @ xA^T     (adapter fused in PSUM)

Layouts: contraction dim (d_in) on SBUF partitions for both matmul operands,
so the host passes x and weight transposed. Output is produced transposed
(y^T per core) and un-transposed on the host during the gather step.

Matmuls run as float32r (TF32) at 1 cycle/row -- 4x faster than plain fp32
on the PE with ~11-bit mantissa precision.
"""

import sys

import numpy as np

sys.path.insert(0, "/opt/trn_rl_repo")

import concourse.bass as bass
import concourse.bacc as bacc
import concourse.mybir as mybir
import concourse.tile as tile

F32 = mybir.dt.float32
F32R = mybir.dt.float32r
P = 128

# Problem dims (hardcoded per the task contract).
BS, SEQ, DIN, DOUT = 8, 2048, 4096, 4096
NSK, RANK = 8, 16
SR = NSK * RANK  # 128 = (skill, rank) pairs, one per partition
N_CORES = 8
EPS = 1e-12


def build_nc(din=DIN, dout=DOUT, seq=SEQ, n_half=2, nchunk=512):
    """Build the single-core Bass program (SPMD: same program on all cores)."""
    th = seq // n_half      # tokens resident in SBUF at a time
    KT = din // P           # contraction k-tiles
    MT = dout // P          # output-feature m-tiles
    NCH = th // nchunk      # token chunks per resident half
    OC = dout // nchunk     # 512-chunks for the B mix

    nc = bacc.Bacc()
    xT_d = nc.declare_dram_parameter("xT", [din, seq], F32R, isOutput=False)
    wT_d = nc.declare_dram_parameter("wT", [din, dout], F32R, isOutput=False)
    bias_d = nc.declare_dram_parameter("biasr", [P, MT], F32, isOutput=False)
    la_d = nc.declare_dram_parameter("lar", [SR, din], F32, isOutput=False)
    lb_d = nc.declare_dram_parameter("lbr", [SR, dout], F32, isOutput=False)
    lrow_d = nc.declare_dram_parameter("lrow", [1, NSK], F32, isOutput=False)
    selwa_d = nc.declare_dram_parameter("selwa", [SR, NSK], F32, isOutput=False)
    selwb_d = nc.declare_dram_parameter("selwb", [SR, NSK], F32, isOutput=False)
    maskj_d = nc.declare_dram_parameter("maskj", [SR, RANK], F32, isOutput=False)
    yT_d = nc.declare_dram_parameter("yT", [dout, seq], F32, isOutput=True)

    xT_r = xT_d[:].rearrange("(kt p) t -> p kt t", p=P)
    wT_r = wT_d[:].rearrange("(kt p) o -> p kt o", p=P)
    yT_r = yT_d[:].rearrange("(mt p) t -> p mt t", p=P)

    with tile.TileContext(nc) as tc:
        with (
            tc.tile_pool(name="const", bufs=1) as pc,
            tc.tile_pool(name="wpool", bufs=2) as pw,
            tc.tile_pool(name="xpool", bufs=1) as px,
            tc.tile_pool(name="opool", bufs=3) as po,
            tc.tile_pool(name="ppm", bufs=4, space="PSUM") as ppm,
            tc.tile_pool(name="ppxa", bufs=2, space="PSUM") as ppxa,
            tc.tile_pool(name="ppmix", bufs=2, space="PSUM") as ppmix,
        ):
            # ---------- constants ----------
            selwa = pc.tile([SR, NSK], F32, name="selwa")
            nc.sync.dma_start(out=selwa[:], in_=selwa_d[:])
            selwb = pc.tile([SR, NSK], F32, name="selwb")
            nc.sync.dma_start(out=selwb[:], in_=selwb_d[:])
            maskj = pc.tile([SR, RANK], F32, name="maskj")
            nc.sync.dma_start(out=maskj[:], in_=maskj_d[:])
            bias_sb = pc.tile([P, MT], F32, name="bias_sb")
            nc.sync.dma_start(out=bias_sb[:], in_=bias_d[:])

            # ---------- routing: normalized sigmoid gate ----------
            # The logits row is broadcast to all partitions and the (tiny)
            # normalization is computed redundantly on each one.
            lrowb = pc.tile([SR, NSK], F32, name="lrowb")
            nc.sync.dma_start(out=lrowb[:], in_=lrow_d[:].to_broadcast((SR, NSK)))
            sig = pc.tile([SR, NSK], F32, name="sig")
            nc.scalar.activation(sig[:], lrowb[:], mybir.ActivationFunctionType.Sigmoid)
            ssum = pc.tile([SR, 1], F32, name="ssum")
            nc.vector.reduce_sum(ssum[:], sig[:], axis=mybir.AxisListType.X)
            nc.vector.tensor_scalar_add(ssum[:], ssum[:], float(EPS))
            rinv = pc.tile([SR, 1], F32, name="rinv")
            nc.vector.reciprocal(rinv[:], ssum[:])
            wbfull = pc.tile([SR, NSK], F32, name="wbfull")
            nc.vector.tensor_scalar_mul(wbfull[:], sig[:], rinv[:])

            # per-partition gate value: wv[p] = w[p // RANK] (A path: / RANK)
            tmpa = pc.tile([SR, NSK], F32, name="tmpa")
            nc.vector.tensor_tensor(tmpa[:], wbfull[:], selwa[:], mybir.AluOpType.mult)
            wva = pc.tile([SR, 1], F32, name="wva")
            nc.vector.reduce_sum(wva[:], tmpa[:], axis=mybir.AxisListType.X)
            tmpb = pc.tile([SR, NSK], F32, name="tmpb")
            nc.vector.tensor_tensor(tmpb[:], wbfull[:], selwb[:], mybir.AluOpType.mult)
            wvb = pc.tile([SR, 1], F32, name="wvb")
            nc.vector.reduce_sum(wvb[:], tmpb[:], axis=mybir.AxisListType.X)

            # ---------- mix LoRA A: A_sb[d, j] = sum_s w_s/RANK * la[s, d, j] ----------
            la = pw.tile([SR, din], F32, tag="w", name="la")
            nc.sync.dma_start(out=la[:], in_=la_d[:])
            nc.vector.tensor_scalar_mul(la[:], la[:], wva[:])
            a_sb = pc.tile([P, KT * RANK], F32R, name="a_sb")
            for kt in range(KT):
                pmix = ppmix.tile([P, RANK], F32, tag="pmix", name=f"pmixa{kt}")
                nc.tensor.matmul(
                    pmix[:], la[:, kt * P:(kt + 1) * P], maskj[:],
                    start=True, stop=True,
                )
                nc.vector.tensor_copy(a_sb[:, kt * RANK:(kt + 1) * RANK], pmix[:])

            # ---------- mix LoRA B: B_sb[r, o] = sum_s w_s * lb[s, r, o] ----------
            b_sb = pc.tile([RANK, dout], F32R, name="b_sb")
            lb = pw.tile([SR, dout], F32, tag="w", name="lb")
            nc.sync.dma_start(out=lb[:], in_=lb_d[:])
            nc.vector.tensor_scalar_mul(lb[:], lb[:], wvb[:])
            for oc in range(OC):
                pmix = ppmix.tile([RANK, nchunk], F32, tag="pmix", name=f"pmixb{oc}")
                nc.tensor.matmul(
                    pmix[:], maskj[:], lb[:, oc * nchunk:(oc + 1) * nchunk],
                    start=True, stop=True,
                )
                nc.vector.tensor_copy(
                    b_sb[:, oc * nchunk:(oc + 1) * nchunk], pmix[:]
                )

            xat = pc.tile([RANK, th], F32R, name="xat")

            # ---------- main: halves of the token dim stay SBUF-resident ----------
            for h in range(n_half):
                xh = px.tile([P, KT, th], F32R, tag="xh", name=f"xh{h}")
                for kt in range(KT):
                    nc.sync.dma_start(
                        out=xh[:, kt, :], in_=xT_r[:, kt, h * th:(h + 1) * th]
                    )

                # xA^T for this half (K on partitions, accumulated over k-tiles)
                for ncn in range(NCH):
                    ts = slice(ncn * nchunk, (ncn + 1) * nchunk)
                    pxa = ppxa.tile([RANK, nchunk], F32, tag="pxa", name=f"pxa{h}_{ncn}")
                    for kt in range(KT):
                        nc.tensor.matmul(
                            pxa[:],
                            a_sb[:, kt * RANK:(kt + 1) * RANK],
                            xh[:, kt, ts],
                            start=(kt == 0), stop=(kt == KT - 1),
                        )
                    nc.vector.tensor_copy(xat[:, ts], pxa[:])

                # dense W @ x^T with the adapter fused as a 33rd accumulation
                for m in range(MT):
                    wm = pw.tile([P, KT, P], F32R, tag="w", name=f"wm{h}_{m}")
                    nc.sync.dma_start(out=wm[:], in_=wT_r[:, :, m * P:(m + 1) * P])
                    for ncn in range(NCH):
                        ts = slice(ncn * nchunk, (ncn + 1) * nchunk)
                        pm = ppm.tile([P, nchunk], F32, tag="pm", name=f"pm{h}_{m}_{ncn}")
                        for kt in range(KT):
                            nc.tensor.matmul(
                                pm[:],
                                wm[:, kt, :],
                                xh[:, kt, ts],
                                start=(kt == 0), stop=False,
                            )
                        nc.tensor.matmul(
                            pm[:],
                            b_sb[:, m * P:(m + 1) * P],
                            xat[:, ts],
                            start=False, stop=True,
                        )
                        o_sb = po.tile([P, nchunk], F32, tag="o", name=f"o{h}_{m}_{ncn}")
                        nc.vector.tensor_scalar_add(o_sb[:], pm[:], bias_sb[:, m:m + 1])
                        nc.sync.dma_start(
                            out=yT_r[:, m, h * th + ncn * nchunk:h * th + (ncn + 1) * nchunk],
                            in_=o_sb[:],
                        )
    return nc


def _round_fp32r(a):
    """Round fp32 to the fp32r (TF32-like) bit pattern: round-half-even at
    mantissa bit 12. Matches neuron_dtypes cast_fp32_to_fp32r bit-exactly."""
    u = np.ascontiguousarray(a, dtype=np.float32).view(np.uint32)
    r = (u.astype(np.uint64) + 0x7FF + ((u >> 12) & 1)) & 0xFFFFF000
    return r.astype(np.uint32).view(np.float32)


def host_prep(x, weight, bias, module_logits, lora_a, lora_b, task_ids,
              din=DIN, dout=DOUT):
    """Shard + lay out the full inputs into per-core in_maps."""
    x = np.asarray(x, dtype=np.float32)
    weight = np.asarray(weight, dtype=np.float32)
    bias = np.asarray(bias, dtype=np.float32)
    module_logits = np.asarray(module_logits, dtype=np.float32)
    lora_a = np.asarray(lora_a, dtype=np.float32)
    lora_b = np.asarray(lora_b, dtype=np.float32)
    task_ids = np.asarray(task_ids).astype(np.int64)

    mt = dout // P
    wT = _round_fp32r(np.ascontiguousarray(weight.T))         # (din, dout)
    bias_r = np.ascontiguousarray(bias.reshape(mt, P).T)      # (P, MT)
    # la_r[(s, j), d] = lora_a[0, s, d, j]
    la_r = np.ascontiguousarray(
        lora_a[0].transpose(0, 2, 1).reshape(SR, din))
    # lb_r[(s, r), o] = lora_b[0, s, r, o]
    lb_r = np.ascontiguousarray(lora_b[0].reshape(SR, dout))
    selwa = np.repeat(np.eye(NSK, dtype=np.float32) / RANK, RANK, axis=0)
    selwb = np.repeat(np.eye(NSK, dtype=np.float32), RANK, axis=0)
    maskj = np.tile(np.eye(RANK, dtype=np.float32), (NSK, 1))

    in_maps = []
    for c in range(x.shape[0]):
        in_maps.append({
            "xT": _round_fp32r(np.ascontiguousarray(x[c].T)),
            "wT": wT,
            "biasr": bias_r,
            "lar": la_r,
            "lbr": lb_r,
            "lrow": np.ascontiguousarray(
                module_logits[task_ids[c]].reshape(1, NSK)),
            "selwa": selwa,
            "selwb": selwb,
            "maskj": maskj,
        })
    return in_maps


_NC_CACHE = {}


def _get_nc():
    if "nc" not in _NC_CACHE:
        nc = build_nc()
        nc.finalize()  # Bacc.finalize: runs the pass pipeline (wait splitting etc.)
        _NC_CACHE["nc"] = nc
    return _NC_CACHE["nc"]


def _ensure_ntff_hook():
    """The agent image's antenv lacks axon_hooks; synthesize it so
    run_bass_kernel_spmd(trace=True) can find the NTFF profile hook."""
    import types

    try:
        from antenv.axon_hooks import get_axon_ntff_profile_hook  # noqa: F401
        return
    except ImportError:
        pass
    import antenv

    mod = types.ModuleType("antenv.axon_hooks")
    mod._hook = None

    def set_axon_ntff_profile_hook(h):
        mod._hook = h

    def get_axon_ntff_profile_hook():
        return mod._hook

    mod.set_axon_ntff_profile_hook = set_axon_ntff_profile_hook
    mod.get_axon_ntff_profile_hook = get_axon_ntff_profile_hook
    sys.modules["antenv.axon_hooks"] = mod
    antenv.axon_hooks = mod

    so_path = "/opt/axon/libaxon_pjrt.so"
    try:
        from trn_agent_boot.trn_boot import _ntff_profile_via_ctypes

        mod._hook = _ntff_profile_via_ctypes(so_path)
    except Exception as e:  # degrade to no tracing
        print(f"ntff hook unavailable: {e}", flush=True)


def run(trace=False, **inputs):
    """Run on the 8 NeuronCores; returns (full_output, BassKernelResults)."""
    from concourse.bass_utils import run_bass_kernel_spmd

    if trace:
        _ensure_ntff_hook()
    in_maps = host_prep(**inputs)
    nc = _get_nc()
    res = run_bass_kernel_spmd(nc, in_maps, list(range(N_CORES)), trace=trace)
    out = np.empty((BS, SEQ, DOUT), dtype=np.float32)
    for c in range(BS):
        out[c] = res.results[c]["yT"].T
    return out, res


def kernel(**inputs):
    out, _ = run(trace=False, **inputs)
    return out


# revision 14
# speedup vs baseline: 1.1780x; 1.1780x over previous
"""PolyLoRALinear Trainium2 kernel (8-core SPMD, data-parallel over batch).

Per-core program (core c handles batch c):
  routing: w = sigmoid(module_logits[task_id]); w /= (sum(w) + eps)
  A_mix(d,16) = sum_s w_s * lora_a[s], B_mix(16,o) = sum_s w_s * lora_b[s]
  xA^T(16,t)  = A_mix^T @ x^T                       (1/16 folded into A path)
  y^T(o,t)    = W @ x^T + bias + B_mix^T @ xA^T     (adapter fused in PSUM)

Layouts: contraction dim (d_in) on SBUF partitions for both matmul operands,
so the host passes x and weight transposed. Output is produced transposed
(y^T per core) and un-transposed on the host during the gather step.

Matmuls run as float32r (TF32) at 1 cycle/row -- 4x faster than plain fp32
on the PE with ~11-bit mantissa precision.
"""

import sys

import numpy as np

sys.path.insert(0, "/opt/trn_rl_repo")

import concourse.bass as bass
import concourse.bacc as bacc
import concourse.mybir as mybir
import concourse.tile as tile

F32 = mybir.dt.float32
F32R = mybir.dt.float32r
P = 128

# Problem dims (hardcoded per the task contract).
BS, SEQ, DIN, DOUT = 8, 2048, 4096, 4096
NSK, RANK = 8, 16
SR = NSK * RANK  # 128 = (skill, rank) pairs, one per partition
N_CORES = 8
EPS = 1e-12


def build_nc(din=DIN, dout=DOUT, seq=SEQ, n_half=2, nchunk=512):
    """Build the single-core Bass program (SPMD: same program on all cores)."""
    th = seq // n_half      # tokens resident in SBUF at a time
    KT = din // P           # contraction k-tiles
    MT = dout // P          # output-feature m-tiles
    NCH = th // nchunk      # token chunks per resident half
    OC = dout // nchunk     # 512-chunks for the B mix

    nc = bacc.Bacc()
    xT_d = nc.declare_dram_parameter("xT", [din, seq], F32R, isOutput=False)
    wT_d = nc.declare_dram_parameter("wT", [din, dout], F32R, isOutput=False)
    bias_d = nc.declare_dram_parameter("biasr", [P, MT], F32, isOutput=False)
    la_d = nc.declare_dram_parameter("lar", [SR, din], F32, isOutput=False)
    lb_d = nc.declare_dram_parameter("lbr", [SR, dout], F32, isOutput=False)
    lrow_d = nc.declare_dram_parameter("lrow", [1, NSK], F32, isOutput=False)
    selwa_d = nc.declare_dram_parameter("selwa", [SR, NSK], F32, isOutput=False)
    selwb_d = nc.declare_dram_parameter("selwb", [SR, NSK], F32, isOutput=False)
    maskj_d = nc.declare_dram_parameter("maskj", [SR, RANK], F32, isOutput=False)
    yT_d = nc.declare_dram_parameter("yT", [dout, seq], F32, isOutput=True)

    xT_r = xT_d[:].rearrange("(kt p) t -> p kt t", p=P)
    wT_r = wT_d[:].rearrange("(kt p) o -> p kt o", p=P)
    yT_r = yT_d[:].rearrange("(mt p) t -> p mt t", p=P)

    with tile.TileContext(nc) as tc:
        with (
            tc.tile_pool(name="const", bufs=1) as pc,
            tc.tile_pool(name="wpool", bufs=2) as pw,
            tc.tile_pool(name="xpool", bufs=1) as px,
            tc.tile_pool(name="opool", bufs=3) as po,
            tc.tile_pool(name="ppm", bufs=4, space="PSUM") as ppm,
            tc.tile_pool(name="ppxa", bufs=2, space="PSUM") as ppxa,
            tc.tile_pool(name="ppmix", bufs=2, space="PSUM") as ppmix,
        ):
            # ---------- constants ----------
            selwa = pc.tile([SR, NSK], F32, name="selwa")
            nc.sync.dma_start(out=selwa[:], in_=selwa_d[:])
            selwb = pc.tile([SR, NSK], F32, name="selwb")
            nc.sync.dma_start(out=selwb[:], in_=selwb_d[:])
            maskj = pc.tile([SR, RANK], F32, name="maskj")
            nc.sync.dma_start(out=maskj[:], in_=maskj_d[:])
            bias_sb = pc.tile([P, MT], F32, name="bias_sb")
            nc.sync.dma_start(out=bias_sb[:], in_=bias_d[:])

            # ---------- routing: normalized sigmoid gate ----------
            # The logits row is broadcast to all partitions and the (tiny)
            # normalization is computed redundantly on each one.
            lrowb = pc.tile([SR, NSK], F32, name="lrowb")
            nc.sync.dma_start(out=lrowb[:], in_=lrow_d[:].to_broadcast((SR, NSK)))
            sig = pc.tile([SR, NSK], F32, name="sig")
            nc.scalar.activation(sig[:], lrowb[:], mybir.ActivationFunctionType.Sigmoid)
            ssum = pc.tile([SR, 1], F32, name="ssum")
            nc.vector.reduce_sum(ssum[:], sig[:], axis=mybir.AxisListType.X)
            nc.vector.tensor_scalar_add(ssum[:], ssum[:], float(EPS))
            rinv = pc.tile([SR, 1], F32, name="rinv")
            nc.vector.reciprocal(rinv[:], ssum[:])
            wbfull = pc.tile([SR, NSK], F32, name="wbfull")
            nc.vector.tensor_scalar_mul(wbfull[:], sig[:], rinv[:])

            # per-partition gate value: wv[p] = w[p // RANK] (A path: / RANK)
            tmpa = pc.tile([SR, NSK], F32, name="tmpa")
            nc.vector.tensor_tensor(tmpa[:], wbfull[:], selwa[:], mybir.AluOpType.mult)
            wva = pc.tile([SR, 1], F32, name="wva")
            nc.vector.reduce_sum(wva[:], tmpa[:], axis=mybir.AxisListType.X)
            tmpb = pc.tile([SR, NSK], F32, name="tmpb")
            nc.vector.tensor_tensor(tmpb[:], wbfull[:], selwb[:], mybir.AluOpType.mult)
            wvb = pc.tile([SR, 1], F32, name="wvb")
            nc.vector.reduce_sum(wvb[:], tmpb[:], axis=mybir.AxisListType.X)

            # ---------- mix LoRA A: A_sb[d, j] = sum_s w_s/RANK * la[s, d, j] ----------
            la = pw.tile([SR, din], F32, tag="w", name="la")
            nc.sync.dma_start(out=la[:], in_=la_d[:])
            nc.vector.tensor_scalar_mul(la[:], la[:], wva[:])
            a_sb = pc.tile([P, KT * RANK], F32R, name="a_sb")
            for kt in range(KT):
                pmix = ppmix.tile([P, RANK], F32, tag="pmix", name=f"pmixa{kt}")
                nc.tensor.matmul(
                    pmix[:], la[:, kt * P:(kt + 1) * P], maskj[:],
                    start=True, stop=True,
                )
                nc.vector.tensor_copy(a_sb[:, kt * RANK:(kt + 1) * RANK], pmix[:])

            # ---------- mix LoRA B: B_sb[r, o] = sum_s w_s * lb[s, r, o] ----------
            b_sb = pc.tile([RANK, dout], F32R, name="b_sb")
            lb = pw.tile([SR, dout], F32, tag="w", name="lb")
            nc.sync.dma_start(out=lb[:], in_=lb_d[:])
            nc.vector.tensor_scalar_mul(lb[:], lb[:], wvb[:])
            for oc in range(OC):
                pmix = ppmix.tile([RANK, nchunk], F32, tag="pmix", name=f"pmixb{oc}")
                nc.tensor.matmul(
                    pmix[:], maskj[:], lb[:, oc * nchunk:(oc + 1) * nchunk],
                    start=True, stop=True,
                )
                nc.vector.tensor_copy(
                    b_sb[:, oc * nchunk:(oc + 1) * nchunk], pmix[:]
                )

            xat = pc.tile([RANK, th], F32R, name="xat")

            # ---------- main: halves of the token dim stay SBUF-resident ----------
            for h in range(n_half):
                xh = px.tile([P, KT, th], F32R, tag="xh", name=f"xh{h}")
                for kt in range(KT):
                    nc.sync.dma_start(
                        out=xh[:, kt, :], in_=xT_r[:, kt, h * th:(h + 1) * th]
                    )

                # xA^T for this half. NCH accumulation groups advance in
                # lockstep so consecutive matmuls share the same stationary
                # operand (one weight load serves NCH matmuls).
                chunks = [slice(ncn * nchunk, (ncn + 1) * nchunk) for ncn in range(NCH)]
                pxas = [
                    ppxa.tile([RANK, nchunk], F32, tag="pxa", name=f"pxa{h}_{ncn}")
                    for ncn in range(NCH)
                ]
                for kt in range(KT):
                    for ncn in range(NCH):
                        nc.tensor.matmul(
                            pxas[ncn][:],
                            a_sb[:, kt * RANK:(kt + 1) * RANK],
                            xh[:, kt, chunks[ncn]],
                            start=(kt == 0), stop=(kt == KT - 1),
                        )
                for ncn in range(NCH):
                    nc.vector.tensor_copy(xat[:, chunks[ncn]], pxas[ncn][:])

                # dense W @ x^T with the adapter fused as the last accumulation
                for m in range(MT):
                    wm = pw.tile([P, KT, P], F32R, tag="w", name=f"wm{h}_{m}")
                    nc.sync.dma_start(out=wm[:], in_=wT_r[:, :, m * P:(m + 1) * P])
                    pms = [
                        ppm.tile([P, nchunk], F32, tag="pm", name=f"pm{h}_{m}_{ncn}")
                        for ncn in range(NCH)
                    ]
                    for kt in range(KT):
                        for ncn in range(NCH):
                            nc.tensor.matmul(
                                pms[ncn][:],
                                wm[:, kt, :],
                                xh[:, kt, chunks[ncn]],
                                start=(kt == 0), stop=False,
                            )
                    for ncn in range(NCH):
                        nc.tensor.matmul(
                            pms[ncn][:],
                            b_sb[:, m * P:(m + 1) * P],
                            xat[:, chunks[ncn]],
                            start=False, stop=True,
                        )
                        o_sb = po.tile([P, nchunk], F32, tag="o", name=f"o{h}_{m}_{ncn}")
                        nc.vector.tensor_scalar_add(o_sb[:], pms[ncn][:], bias_sb[:, m:m + 1])
                        nc.sync.dma_start(
                            out=yT_r[:, m, h * th + ncn * nchunk:h * th + (ncn + 1) * nchunk],
                            in_=o_sb[:],
                        )
    return nc


def _round_fp32r(a):
    """Round fp32 to the fp32r (TF32-like) bit pattern: round-half-even at
    mantissa bit 12. Matches neuron_dtypes cast_fp32_to_fp32r bit-exactly."""
    u = np.ascontiguousarray(a, dtype=np.float32).view(np.uint32)
    r = (u.astype(np.uint64) + 0x7FF + ((u >> 12) & 1)) & 0xFFFFF000
    return r.astype(np.uint32).view(np.float32)


def host_prep(x, weight, bias, module_logits, lora_a, lora_b, task_ids,
              din=DIN, dout=DOUT):
    """Shard + lay out the full inputs into per-core in_maps."""
    x = np.asarray(x, dtype=np.float32)
    weight = np.asarray(weight, dtype=np.float32)
    bias = np.asarray(bias, dtype=np.float32)
    module_logits = np.asarray(module_logits, dtype=np.float32)
    lora_a = np.asarray(lora_a, dtype=np.float32)
    lora_b = np.asarray(lora_b, dtype=np.float32)
    task_ids = np.asarray(task_ids).astype(np.int64)

    mt = dout // P
    wT = _round_fp32r(np.ascontiguousarray(weight.T))         # (din, dout)
    bias_r = np.ascontiguousarray(bias.reshape(mt, P).T)      # (P, MT)
    # la_r[(s, j), d] = lora_a[0, s, d, j]
    la_r = np.ascontiguousarray(
        lora_a[0].transpose(0, 2, 1).reshape(SR, din))
    # lb_r[(s, r), o] = lora_b[0, s, r, o]
    lb_r = np.ascontiguousarray(lora_b[0].reshape(SR, dout))
    selwa = np.repeat(np.eye(NSK, dtype=np.float32) / RANK, RANK, axis=0)
    selwb = np.repeat(np.eye(NSK, dtype=np.float32), RANK, axis=0)
    maskj = np.tile(np.eye(RANK, dtype=np.float32), (NSK, 1))

    in_maps = []
    for c in range(x.shape[0]):
        in_maps.append({
            "xT": _round_fp32r(np.ascontiguousarray(x[c].T)),
            "wT": wT,
            "biasr": bias_r,
            "lar": la_r,
            "lbr": lb_r,
            "lrow": np.ascontiguousarray(
                module_logits[task_ids[c]].reshape(1, NSK)),
            "selwa": selwa,
            "selwb": selwb,
            "maskj": maskj,
        })
    return in_maps


_NC_CACHE = {}


def _enable_ldw_opt():
    """Let walrus dedupe consecutive identical LDWEIGHTS. The main loop is
    ordered so each weight tile serves NCH back-to-back matmuls; fp32r
    weight loads (~213ns for 128 fp32 columns) otherwise gate the PE."""
    from concourse import bass_utils as bu

    if getattr(bu, "_ldw_opt_patched", False):
        return
    orig = bu.run_command

    def patched(cmd, *a, **kw):
        cmd = [c.replace("--enable-ldw-opt=false", "--enable-ldw-opt=true")
               if isinstance(c, str) else c for c in cmd]
        return orig(cmd, *a, **kw)

    bu.run_command = patched
    bu._ldw_opt_patched = True


def _get_nc():
    if "nc" not in _NC_CACHE:
        nc = build_nc()
        nc.finalize()  # Bacc.finalize: runs the pass pipeline (wait splitting etc.)
        _NC_CACHE["nc"] = nc
    return _NC_CACHE["nc"]


def _ensure_ntff_hook():
    """The agent image's antenv lacks axon_hooks; synthesize it so
    run_bass_kernel_spmd(trace=True) can find the NTFF profile hook."""
    import types

    try:
        from antenv.axon_hooks import get_axon_ntff_profile_hook  # noqa: F401
        return
    except ImportError:
        pass
    import antenv

    mod = types.ModuleType("antenv.axon_hooks")
    mod._hook = None

    def set_axon_ntff_profile_hook(h):
        mod._hook = h

    def get_axon_ntff_profile_hook():
        return mod._hook

    mod.set_axon_ntff_profile_hook = set_axon_ntff_profile_hook
    mod.get_axon_ntff_profile_hook = get_axon_ntff_profile_hook
    sys.modules["antenv.axon_hooks"] = mod
    antenv.axon_hooks = mod

    so_path = "/opt/axon/libaxon_pjrt.so"
    try:
        from trn_agent_boot.trn_boot import _ntff_profile_via_ctypes

        mod._hook = _ntff_profile_via_ctypes(so_path)
    except Exception as e:  # degrade to no tracing
        print(f"ntff hook unavailable: {e}", flush=True)


def run(trace=False, **inputs):
    """Run on the 8 NeuronCores; returns (full_output, BassKernelResults)."""
    from concourse.bass_utils import run_bass_kernel_spmd

    if trace:
        _ensure_ntff_hook()
    _enable_ldw_opt()
    in_maps = host_prep(**inputs)
    nc = _get_nc()
    res = run_bass_kernel_spmd(nc, in_maps, list(range(N_CORES)), trace=trace)
    out = np.empty((BS, SEQ, DOUT), dtype=np.float32)
    for c in range(BS):
        out[c] = res.results[c]["yT"].T
    return out, res


def kernel(**inputs):
    out, _ = run(trace=False, **inputs)
    return out
